# revision 42
# baseline (speedup 1.0000x reference)
"""Trainium2 Bass kernel for nn_ExtractPatchesPositionLayer.

Reference semantics: per image b, bilinear-translate the (522,522,1) padded
object by t = -positions[b] (tfa.translate: out(y,x) = img(y+py, x+px),
zero fill outside), then center-crop 5px -> (512,512,1).

Because the shift is constant per image, floor/frac of the offset give an
integer window start (A,B) into the (zero-margin-padded) image plus four
constant bilinear corner weights:

    out[r, j] = c00*W[r, j] + c01*W[r, j+1] + c10*W[r+1, j] + c11*W[r+1, j+1]
    W[r, c] = pp[A+r, B+c]

Layout trick: SBUF partition p holds FOUR consecutive padded-image rows
(A+4p .. A+4p+3, +1 elem) as ONE contiguous DRAM span (4*wpad+1 elements, a
single ~8.4 KB line-rate DMA descriptor per partition).  The shared
horizontal lerp h = (1-wx)*wt + wx*wt[+1] is computed once over the whole
span in RATIO form (one DVE fused madd: g = wt + rx*wt[+1], rx = wx/(1-wx);
all CONTIGUOUS free-dim APs -- DVE runs flat APs at ~2x the rate of strided
3D ones).  The vertical lerp is partition-local (m = g + ry*g[+wpad]) except
each partition's LAST row pair, whose g row 4 == next partition's g row 0:
the otherwise-idle PE recovers it with a shift-matrix matmul (zero last
column, so ps[127,:]=0 stays defined) that the DVE madd reads straight from
PSUM.  The combined scale S=(1-wx)(1-wy) is applied LAST by ACT, which runs
strided APs at full rate and therefore writes a COMPACT 512-wide output
tile: 4 consecutive y rows per partition = one contiguous 8 KB descriptor,
no write junk, no host trim.  The very last output row (needs input row
A+512, outside the spans) is patched on host -- O(B*N) work.  Ratio form is
numerically safe here: 1-wx, 1-wy in (0,1], and the big term dominates both
g and the output, keeping error at the output's ulp scale.

DMA routing (hard-won trace facts):
  * inputs: dynamic HWDGE on the SP ring (runtime reg offsets; descriptors
    spread over all 16 SDMA engines by dest SBUF partition).
  * outputs: SWDGE via gpsimd -- HWDGE sends every SBUF->HBM descriptor to
    SDMA engine 0 (1.4 ms serialized); SWDGE's CounterMachine spreads them.
    8+ KB descriptors avoid SWDGE's 8-byte stub-packet flood seen at 2 KB.
Sharding: batch 256 -> 32 images x 8 cores, embarrassingly parallel.
Measured: 1426 us (baseline banded-matmul PE kernel) -> 181 us best-state;
the 16 SDMA engines (~165 us busy each, work-conserving at ~100% mid-run)
are the binding resource (~400+ GB/s aggregate HBM traffic, past the
documented 358 GB/s per-core); DVE ~155 us.  Device power-state drift adds
up to ~12% run-to-run.
"""

from dataclasses import dataclass

import numpy as np

import concourse.bacc as bacc
import concourse.bass as bass
import concourse.mybir as mybir
import concourse.tile as tile
from concourse.bass_utils import run_bass_kernel_spmd


@dataclass(frozen=True)
class Cfg:
    bpc: int      # images per core
    n: int        # output height/width
    wpad: int     # padded input height/width (with zero margin)
    xlen: int     # flat padded-input length per core (incl. tail pad)

    @property
    def wrow(self):  # output rows per partition
        return self.n // 128

    @property
    def span(self):  # elements DMA'd per partition (WR rows + 1)
        return self.wrow * self.wpad + 1


def build_nc(cfg: Cfg) -> bass.Bass:
    BPC, N, WPAD = cfg.bpc, cfg.n, cfg.wpad
    WR = cfg.wrow
    SPAN = cfg.span
    WIDE = WR * WPAD  # full-width output row block per partition
    XLEN = cfg.xlen
    f32 = mybir.dt.float32
    i32 = mybir.dt.int32
    MUL = mybir.AluOpType.mult
    ADD = mybir.AluOpType.add

    nc = bacc.Bacc("TRN2", target_bir_lowering=False, debug=False)
    x_d = nc.declare_dram_parameter("x", [1, XLEN], f32, isOutput=False)
    offs_d = nc.declare_dram_parameter("offs", [1, BPC], i32, isOutput=False)
    wmat_d = nc.declare_dram_parameter("wmat", [BPC, 128, 4], f32, isOutput=False)
    smat_d = nc.declare_dram_parameter("smat", [128, 128], f32, isOutput=False)
    y_d = nc.declare_dram_parameter("y", [BPC, N, N], f32, isOutput=True)

    with tile.TileContext(nc) as tc:
        with (
            tc.tile_pool(name="const", bufs=1) as constp,
            tc.tile_pool(name="win", bufs=9) as winp,
            tc.tile_pool(name="hp", bufs=5) as hp,
            tc.tile_pool(name="mp", bufs=4) as mp,
            tc.tile_pool(name="op", bufs=5) as op,
            tc.tile_pool(name="psp", bufs=8, space="PSUM") as psp,
        ):
            # consts ride the ACT HWDGE ring so the SP ring's FIFO head is
            # the first window DMA (shaves the pipeline ramp)
            wmat_sb = constp.tile([128, BPC * 4], f32, tag="wmat")
            nc.scalar.dma_start(
                wmat_sb[:].rearrange("p (i q) -> p i q", q=4),
                wmat_d[:, :, :].transpose([1, 0, 2]),
            )
            offs_sb = constp.tile([1, BPC], i32, tag="offs")
            nc.scalar.dma_start(offs_sb[:], offs_d[:, :])
            smat_sb = constp.tile([128, 128], f32, tag="smat")
            nc.scalar.dma_start(smat_sb[:], smat_d[:, :])

            regs = [nc.alloc_register(mybir.EngineType.SP, f"dynoff_{k}")
                    for k in range(min(16, BPC))]
            svs = [nc.snap(r, donate=True, min_val=0, max_val=XLEN - 1)
                   for r in regs]
            nreg = len(regs)

            for i in range(BPC):
                k = i % nreg
                nc.sync.reg_load(regs[k], offs_sb[0:1, i: i + 1])
                wt = winp.tile([128, SPAN], f32, tag="wt")
                nc.sync.dma_start(
                    wt[:],
                    bass.AP(x_d, svs[k], [[WR * WPAD, 128], [1, SPAN]]),
                )
                # all operands are full-width CONTIGUOUS slices (junk
                # between rows is computed and trimmed on host): DVE runs
                # flat APs at full rate, strided 3D ones at half rate.
                # Shared horizontal lerp h over the whole span, then a
                # partition-local vertical lerp of h against h-shifted-by-
                # one-row: 4 passes total (2 ACT muls + 2 DVE madds).
                # ratio-form lerp, scale applied LAST by ACT (which runs
                # strided APs at full rate) into a COMPACT 512-wide output:
                #   g = wt + rx*wt[+1];  m = g + ry*g[+wpad];  y = S*m
                rx = wmat_sb[:, 4 * i + 0: 4 * i + 1]
                ry = wmat_sb[:, 4 * i + 1: 4 * i + 2]
                sc = wmat_sb[:, 4 * i + 2: 4 * i + 3]

                HL = SPAN - 1  # = WIDE: g rows 0..WR-1
                W3 = (WR - 1) * WPAD
                g = hp.tile([128, HL], f32, tag="g")
                m = mp.tile([128, W3 + N], f32, tag="m")
                oc = op.tile([128, WR * N], f32, tag="oc")
                ps = psp.tile([128, N], f32, tag="ps")

                nc.vector.scalar_tensor_tensor(g[:], wt[:, 1:HL + 1], rx,
                                               wt[:, 0:HL], MUL, ADD)
                # g row WR (= next partition's g row 0) via idle-PE partition
                # shift: ps[q, j] = g[q+1, j].  Global row N-1+1 has no next
                # partition -- that one output row is patched on host.
                nc.tensor.matmul(out=ps[:], lhsT=smat_sb[:, :],
                                 rhs=g[:, 0:N], start=True, stop=True)
                nc.vector.scalar_tensor_tensor(
                    m[:, 0:W3], g[:, WPAD:WR * WPAD], ry,
                    g[:, 0:W3], MUL, ADD)
                # smat column 127 is all-zero, so ps[127,:] = 0 and
                # partition 127 passes g through (host-patched row anyway)
                nc.vector.scalar_tensor_tensor(
                    m[:, W3:W3 + N], ps[:], ry,
                    g[:, W3:W3 + N], MUL, ADD)
                nc.scalar.mul(
                    oc[:, 0:(WR - 1) * N].rearrange("p (u j) -> p u j", j=N),
                    m[:, 0:W3].rearrange("p (u j) -> p u j", j=WPAD)[:, :, 0:N],
                    sc)
                nc.scalar.mul(oc[:, (WR - 1) * N:WR * N], m[:, W3:W3 + N], sc)

                nc.gpsimd.dma_start(
                    bass.AP(y_d, i * (N * N), [[WR * N, 128], [1, WR * N]]),
                    oc[:],
                )
    nc.compile()
    return nc


def host_prep(padded: np.ndarray, positions: np.ndarray, n_cores: int):
    """Shard + build metadata. padded: (B, npad, npad) f32, positions: (B, 2)."""
    B, npad, _ = padded.shape
    n = npad - 10
    bpc = B // n_cores

    px = positions[:, 0].astype(np.float32)
    py = positions[:, 1].astype(np.float32)
    fy = np.floor(py)
    fx = np.floor(px)
    ay = (5 + fy).astype(np.int64)
    ax = (5 + fx).astype(np.int64)
    wy = (py - fy).astype(np.float32)
    wx = (px - fx).astype(np.float32)

    m_lo = int(max(0, -min(ay.min(), ax.min())))
    m_hi = int(max(0, max(ay.max(), ax.max()) + n + 1 - npad))
    wpad = npad + m_lo + m_hi

    pp = np.zeros((B, wpad, wpad), dtype=np.float32)
    pp[:, m_lo:m_lo + npad, m_lo:m_lo + npad] = padded

    A = ay + m_lo
    Bc = ax + m_lo
    base = (np.arange(B, dtype=np.int64) % bpc) * (wpad * wpad)
    off = base + A * wpad + Bc

    wr = n // 128
    span = wr * wpad + 1
    # flat length incl. tail so the last image's strided span stays in bounds
    need = int(off.max()) + 127 * wr * wpad + span
    xlen = max(bpc * wpad * wpad, need)

    cfg = Cfg(bpc=bpc, n=n, wpad=wpad, xlen=xlen)

    smat = np.zeros((128, 128), dtype=np.float32)
    for m in range(127):
        smat[m + 1, m] = 1.0  # ps[m, j] = sum_k smat[k, m] g[k, j] = g[m+1, j]
    # column 127 stays zero: ps[127,:] = 0 (that row is host-patched)

    # host-side fixup for the last output row (needs input row A+n, which the
    # 4-row spans don't load)
    ar = np.arange(B)[:, None]
    ci = Bc[:, None] + np.arange(n + 1)[None, :]
    r0 = pp[ar, (A + n - 1)[:, None], ci]  # (B, n+1)
    r1 = pp[ar, (A + n)[:, None], ci]
    h0r = (1 - wx)[:, None] * r0[:, :n] + wx[:, None] * r0[:, 1:]
    h1r = (1 - wx)[:, None] * r1[:, :n] + wx[:, None] * r1[:, 1:]
    last_row = ((1 - wy)[:, None] * h0r + wy[:, None] * h1r).astype(np.float32)

    in_maps = []
    for cidx in range(n_cores):
        sl = slice(cidx * bpc, (cidx + 1) * bpc)
        flat = np.zeros((1, xlen), dtype=np.float32)
        flat[0, :bpc * wpad * wpad] = pp[sl].reshape(-1)
        offs = off[sl].astype(np.int32).reshape(1, bpc)
        wmat = np.empty((bpc, 128, 4), dtype=np.float32)
        wmat[:, :, 0] = (wx / (1 - wx))[sl][:, None]
        wmat[:, :, 1] = (wy / (1 - wy))[sl][:, None]
        wmat[:, :, 2] = ((1 - wx) * (1 - wy))[sl][:, None]
        wmat[:, :, 3] = 0.0
        in_maps.append({"x": flat, "offs": offs, "wmat": wmat, "smat": smat})
    return cfg, in_maps, last_row


N_CORES = 8
_nc_cache: dict = {}


def kernel(padded_obj: np.ndarray, positions: np.ndarray) -> np.ndarray:
    padded_obj = np.asarray(padded_obj)
    positions = np.asarray(positions)
    B, npad, _, C = padded_obj.shape
    cfg, in_maps, last_row = host_prep(
        padded_obj.reshape(B, npad, npad).astype(np.float32, copy=False),
        positions, N_CORES)

    nc = _nc_cache.get(cfg)
    if nc is None:
        nc = build_nc(cfg)
        _nc_cache[cfg] = nc

    res = run_bass_kernel_spmd(nc, in_maps, core_ids=list(range(N_CORES)))
    out = np.concatenate([r["y"][:, :, :cfg.n] for r in res.results], axis=0)
    out = np.ascontiguousarray(out)
    out[:, cfg.n - 1, :] = last_row
    return out.reshape(B, cfg.n, cfg.n, 1)
